# revision 75
# baseline (speedup 1.0000x reference)
"""Trainium2 Bass kernel for the quantized BasicBlock (conv3x3/s2 + fakequant + conv3x3/s1 + fakequant).

Sharding: data-parallel over batch across 8 cores (8 images each), weights replicated.

Device math (per core, B=8):
  conv1: implicit GEMM, 9 taps x 2 ci-blocks, input as fp16 (11-bit significand,
         rel err ~4e-3 on final output vs 2e-2 budget), integer-valued fp16 weights
         (exact), fp32 PSUM accum.
  act1:  v = P1*(s_w1/s_a1) + bq1/s_a1; y = clip(rne(v), -128, 127) via the fp32
         magic-number trick on the DVE; y stored as integer-valued bf16 into a
         zero-padded [16x16] layout for conv2 (plus a 1-col-shifted copy, see below).
  conv2: exact integer bf16 GEMM, 9 taps x 4 ci-blocks.
  act2:  v2 = P2*(s_a1*s_w2/s_a2) + bq2/s_a2; out = clip(rne(v2), -128, 127) * s_a2.

Stride-2 conv1 is handled by a host-side phase split into 2x2 parity planes so each
tap reads a stride-1 14x14 window of one plane.

Alignment: the PE streams the moving operand ~14% slower when its base address is
not 4B-aligned. With 2-byte elements, tap windows at odd column offsets are slow.
Fix: every buffer is stored so all tap windows start at EVEN element offsets --
x planes come in 6 variants (2 normal + 4 one-col-shifted) prepared on host, and
act1 is written twice (normal + 1-col-shifted copy done by the idle Scalar engine).

DMAs are split into small per-(tap/plane, ci-block) chunks with their own tiles and
spread across the two HWDGE queues (Sync: x planes, Scalar: weights) in first-use
order, so the matmul stream starts as soon as tap 0's weights + plane arrive.
"""
import os
import sys
from contextlib import ExitStack

import numpy as np
import ml_dtypes

for _p in ("/opt/trn_rl_repo",):
    if _p not in sys.path and os.path.isdir(_p):
        sys.path.insert(0, _p)

import concourse.bacc as bacc
import concourse.tile as tile
import concourse.mybir as mybir
from concourse.bass_utils import run_bass_kernel_spmd

BF16 = ml_dtypes.bfloat16
N_CORES = 8
B_PER = 8           # images per core
MAGIC = float(np.float32(1.5 * 2 ** 23))   # fp32 RNE rounding magic
Alu = mybir.AluOpType
Act = mybir.ActivationFunctionType
dt = mybir.dt

# tap index k in {0,1,2} -> (parity s, window row offset) for the phase planes
_TAP = {0: (1, 0), 1: (0, 1), 2: (1, 1)}

# conv1 taps grouped by x-plane buffer (first-use order for the DMA ramp)
TAP_ORDER = [0, 6, 2, 8, 1, 7, 3, 5, 4]

# x plane buffers: (sr, sc, col offset of data); 0/1 normal sc=1 planes for kx=0
# (window c0=0), 2..5 one-col-shifted planes pl0..pl3 for kx=1/2 (window c0=2)
_XBUFS = [(0, 1, 1), (1, 1, 1), (0, 0, 2), (0, 1, 2), (1, 0, 2), (1, 1, 2)]


def _c1_src(t9):
    """tap -> (x buffer index, row offset r0, col offset c0); c0 always even."""
    ky, kx = divmod(t9, 3)
    sr, r0 = _TAP[ky]
    sc, _ = _TAP[kx]
    if kx == 0:
        return sr, r0, 0
    return 2 + sr * 2 + sc, r0, 2


# buffers 3 and 5 (shifted copies of 0 and 1) are built on-device by GpSimd
# shift-copies; only these 4 go over DMA (dram slot order):
_XDMA = [0, 1, 2, 4]
_XSLOT = {u: s for s, u in enumerate(_XDMA)}


def _phase_planes(x):
    """(B, C, 28, 28) f32 -> (B, C, 4, 15, 16) parity-plane buffers per _XDMA."""
    B, C = x.shape[:2]
    out = np.zeros((B, C, 4, 15, 16), np.float32)
    for s, u in enumerate(_XDMA):
        sr, sc, off = _XBUFS[u]
        out[:, :, s, 1:15, off:off + 14] = x[:, :, sr::2, sc::2]
    return out


def _quant_weights(w):
    """Per-tensor int8 narrow-range fake quant; returns (int-valued f32 weights, scale)."""
    s = np.float32(np.max(np.abs(w))) / np.float32(127.0)
    wq = np.clip(np.round(w / s), -127, 127).astype(np.float32)
    return wq, s


def _w_lhsT(w_int, n_ci_blk):
    """(Cout=512, Cin, 3, 3) int-valued -> (tap 9, ci_blk, 128, 4, 128) bf16 layout."""
    t = w_int.transpose(2, 3, 1, 0)                      # (3, 3, Cin, 512)
    t = t.reshape(9, n_ci_blk, 128, 4, 128)              # (tap, ci_blk, ci_p, co_blk, co)
    return np.ascontiguousarray(t).astype(BF16)


_skip_ldw = [False]
_orig_InstMatmult = mybir.InstMatmult


def _patched_InstMatmult(*a, **kw):
    if _skip_ldw[0]:
        kw.setdefault("ldweights", False)
    return _orig_InstMatmult(*a, **kw)


def build_program(scale1, scale2, out_scale):
    """Build the (per-core SPMD) Bass program with the given fp32 immediates."""
    nc = bacc.Bacc("TRN2", target_bir_lowering=False, debug=False,
                   num_devices=N_CORES)

    mybir.InstMatmult = _patched_InstMatmult
    try:
        return _build_body(nc, scale1, scale2, out_scale)
    finally:
        mybir.InstMatmult = _orig_InstMatmult


def _build_body(nc, scale1, scale2, out_scale):
    NT = 4

    xhi_d = nc.dram_tensor("xhi", (4, 2, 128, B_PER, 15, 16), dt.float16, kind="ExternalInput")
    # w1 packed partition-major with taps pre-ordered by TAP_ORDER
    w1_d = nc.dram_tensor("w1", (128, 9, 2, 4, 128), dt.float16, kind="ExternalInput")
    # conv2 weights as 1D row-Winograd U (4 r-positions x 3 kx taps), fp16 exact
    u2_d = nc.dram_tensor("u2", (4, 128, 12, 4, 128), dt.float16, kind="ExternalInput")
    b1_d = nc.dram_tensor("b1", (128, 4), dt.float32, kind="ExternalInput")
    b2_d = nc.dram_tensor("b2", (128, 4), dt.float32, kind="ExternalInput")
    out_d = nc.dram_tensor("out", (512, B_PER, 196), dt.int8, kind="ExternalOutput")

    def mm(out_ap, w_ap, rhs, start, stop, reuse):
        # reuse=True -> PE keeps the already-loaded stationary weights
        _skip_ldw[0] = reuse
        try:
            nc.tensor.matmul(out_ap, w_ap, rhs, start=start, stop=stop)
        finally:
            _skip_ldw[0] = False

    with tile.TileContext(nc) as tc, ExitStack() as ctx:
        const = ctx.enter_context(tc.tile_pool(name="const", bufs=1))
        psum = ctx.enter_context(tc.tile_pool(name="psum", bufs=8, space="PSUM"))
        tmp = ctx.enter_context(tc.tile_pool(name="tmp", bufs=3))
        outp = ctx.enter_context(tc.tile_pool(name="outp", bufs=3))

        # --- SBUF allocations: one tile per DMA chunk for fine-grained deps ---
        xb = {(u, b): const.tile([128, B_PER, 15, 16], dt.float16,
                                 tag=f"x{u}_{b}", name=f"x{u}_{b}")
              for u in range(6) for b in range(2)}
        # w1 grouped along TAP_ORDER; first two taps get their own chunk so the
        # very first matmul's weights arrive (and are usable, fp16, no cast) ASAP
        W1G = [(0, 1), (1, 2), (2, 4), (4, 6), (6, 9)]
        w1f_g = [const.tile([128, g1 - g0, 2, 4, 128], dt.float16,
                            tag=f"w1f{g0}", name=f"w1f{g0}") for g0, g1 in W1G]

        def w1_ap(t, b, cb):
            p = TAP_ORDER.index(t)
            for gi, (g0, g1) in enumerate(W1G):
                if g0 <= p < g1:
                    return w1f_g[gi][:, p - g0, b, cb, :]

        u2_t = [const.tile([128, 12, 4, 128], dt.float16, tag=f"u2{b}", name=f"u2t{b}")
                for b in range(4)]
        b1_t = const.tile([128, 4], dt.float32, tag="b1")
        b2_t = const.tile([128, 4], dt.float32, tag="b2")
        act_t = const.tile([128, 4, B_PER, 16, 16], dt.bfloat16, tag="act")   # data cols 1..14
        # 1D row-transformed act (V): [cib] -> [128, r4, img, i7, 16cols], fp16
        # exact (|V| <= 255); vs = 1-col-shifted copy for the kx=1 taps
        vt = [const.tile([128, 4, B_PER, 7, 16], dt.float16, tag=f"vt{b}", name=f"vt{b}")
              for b in range(4)]
        vs = [const.tile([128, 4, B_PER, 7, 16], dt.float16, tag=f"vs{b}", name=f"vs{b}")
              for b in range(4)]
        gate2 = const.tile([128, 1], dt.bfloat16, tag="g2")
        wz = const.tile([128, 256], dt.bfloat16, tag="wz")

        # PE warm-up source + conv2 padding zeros on the otherwise idle GpSimd
        nc.vector.memset(wz[:], 0.0)
        nc.gpsimd.memset(act_t[:], 0.0)

        # --- loads interleaved across both HWDGE queues in demand order so the
        # conv1 tap stream (one 8-MM group per ~0.67us) never outruns delivery.
        # Engine order IS execution order (FIFO queues), so everything below is
        # sequenced explicitly by first use. ---
        def lx(q, u, b):
            q(out=xb[(u, b)][:], in_=xhi_d[_XSLOT[u], b])

        sy, sc_q = nc.sync.dma_start, nc.scalar.dma_start
        lx(sy, 1, 0); lx(sy, 1, 1); lx(sy, 4, 0); lx(sy, 0, 0); lx(sy, 2, 0)
        for gi, (g0, g1) in enumerate(W1G):
            sc_q(out=w1f_g[gi][:], in_=w1_d[:, g0:g1])
            if gi == 3:
                lx(sc_q, 4, 1)
        lx(sc_q, 0, 1)
        sc_q(out=b1_t[:], in_=b1_d[:])
        lx(sc_q, 2, 1)

        # DVE: shifted x duplicates (buf5 <- buf1, buf3 <- buf0). The zero pad
        # ring makes a FLAT +1-element shift exactly equal to the per-row
        # column shift, so this is a fast contiguous 2-byte copy.
        def xshift(u_dst, u_src, b, eng):
            df = xb[(u_dst, b)][:].rearrange("p a b c -> p (a b c)")
            sf = xb[(u_src, b)][:].rearrange("p a b c -> p (a b c)")
            if eng == "v":
                nc.vector.tensor_copy(df[:, 1:1920], sf[:, 0:1919])
            else:
                nc.scalar.activation(df[:, 1:1920], sf[:, 0:1919], Act.Copy)

        # first copy on the DVE (free immediately); the rest on the Scalar
        # engine, which finishes its DMA triggers around the time they're due
        xshift(5, 1, 0, "v")
        xshift(5, 1, 1, "s")
        xshift(3, 0, 0, "s")
        xshift(3, 0, 1, "s")

        def quant_chain(dst, src, sc, bias_ap, width=392):
            """dst = clip(rne(src*sc + bias), -128, 127) on the DVE (3 fused ops)."""
            tt = tmp.tile([128, width], dt.float32, tag=f"tt{min(width, 392)}", name="tt")
            nc.vector.tensor_scalar(tt[:], src, sc, bias_ap, op0=Alu.mult, op1=Alu.add)
            nc.vector.tensor_scalar(tt[:], tt[:], MAGIC, MAGIC + 127.0, op0=Alu.add, op1=Alu.min)
            nc.vector.tensor_scalar(dst, tt[:], MAGIC - 128.0, -MAGIC, op0=Alu.max, op1=Alu.add)
            return tt

        # PE warm-up: junk matmuls on the zeroed tile during the input-DMA wait
        # so the HAM clock gate is at full rate when the real stream starts.
        wps = psum.tile([128, 512], dt.float32, tag="ps", name="warmps")
        for i in range(16):
            nc.tensor.matmul(wps[:, 0:256], wz[:, 0:128], wz[:, 0:256],
                             start=True, stop=True)

        # --- conv1 + act1 ---
        def conv1_group(cb, t9, b, ps_list, nts):
            # one stationary weight (t9, b, cb) serving len(nts) matmuls;
            # only the first self-loads the PE array
            u, r0, c0 = _c1_src(t9)
            w_ap = w1_ap(t9, b, cb)
            for i, nt in enumerate(nts):
                rhs = xb[(u, b)][:, 2 * nt:2 * nt + 2, r0:r0 + 14, c0:c0 + 14]
                mm(ps_list[i][:, 0:392], w_ap, rhs,
                   start=(t9 == TAP_ORDER[0] and b == 0),
                   stop=(t9 == TAP_ORDER[-1] and b == 1),
                   reuse=i > 0)

        def act1_store(cb, nt, ps):
            quant_chain(act_t[:, cb, 2 * nt:2 * nt + 2, 1:15, 1:15],
                        ps[:, 0:392], scale1, b1_t[:, cb:cb + 1])

        def v_transform(b, h, eng):
            """Row-stage Winograd transform of act cib b, image half h (DVE),
            then a flat +1-shift copy into vs (pad ring makes it exact)."""
            im = slice(4 * h, 4 * h + 4)
            d = [act_t[:, b, im, k:k + 13:2, :] for k in range(4)]  # rows 2i+k, [4,7,16]
            v = nc.vector
            v.tensor_tensor(vt[b][:, 0, im], d[0], d[2], op=Alu.subtract)
            v.tensor_tensor(vt[b][:, 1, im], d[1], d[2], op=Alu.add)
            v.tensor_tensor(vt[b][:, 2, im], d[2], d[1], op=Alu.subtract)
            v.tensor_tensor(vt[b][:, 3, im], d[1], d[3], op=Alu.subtract)
            for r in range(4):
                df = vs[b][:, r, im].rearrange("p a b c -> p (a b c)")
                sf = vt[b][:, r, im].rearrange("p a b c -> p (a b c)")
                if eng == "v":
                    nc.vector.tensor_copy(df[:, 1:448], sf[:, 0:447])
                else:
                    nc.scalar.activation(df[:, 1:448], sf[:, 0:447], Act.Copy)

        for cb in range(4):
            if cb == 0:
                # tap-major: plane demand spread over the whole group to match
                # the DMA delivery ramp; 8 matmuls per weight load
                ps_n = [psum.tile([128, 512], dt.float32, tag="ps", name="ps")
                        for _ in range(NT)]
                for t9 in TAP_ORDER:
                    for b in range(2):
                        conv1_group(cb, t9, b, ps_n, range(NT))
                for nt in range(NT):
                    act1_store(cb, nt, ps_n[nt])
                # one tiny Scalar copy gates the u2 load past the x/w1 ramp
                nc.scalar.activation(gate2[:], act_t[:, 0, 0, 0, 0:1], Act.Copy)
                for b in range(4):
                    sc_q(out=u2_t[b][:], in_=u2_d[b])
                sc_q(out=b2_t[:], in_=b2_d[:])
            else:
                # nt-pair-major: each bank pair finishes at half-time so its
                # epilogue overlaps the rest; 4 matmuls per weight load
                for half in range(2):
                    nts = [2 * half, 2 * half + 1]
                    ps_p = [psum.tile([128, 512], dt.float32, tag="ps", name="ps")
                            for _ in nts]
                    for t9 in TAP_ORDER:
                        for b in range(2):
                            conv1_group(cb, t9, b, ps_p, nts)
                    for i, nt in enumerate(nts):
                        act1_store(cb, nt, ps_p[i])
                    # V transforms are emitted one block late (vt0 under cb1,
                    # vt1 under cb2, vt2 under cb2's tail) so the DVE never
                    # delays PSUM-bank recycling for the running conv1 block;
                    # cb3 (needed ~1.5us into conv2) runs right behind its own
                    # epilogue on the DVE.
                    if cb == 1:
                        v_transform(0, half, "s")
                    elif cb == 2:
                        v_transform(1, half, "s")
                        if half == 1:
                            v_transform(2, 0, "s")
                            v_transform(2, 1, "s")
                    else:
                        v_transform(3, half, "v")

        # --- conv2 via 1D row-Winograd + act2 ---
        # out rows pairs: even = M0+M1+M2, odd = M1-M2-M3 over the 4 r-banks.
        # Per (cob, half): 4 banks x 12 accumulating MMs over (kx, cib);
        # cib 3 (conv1's last output block) is ordered last so conv2 can start
        # before conv1's tail epilogue + V transform fully drain.
        SLOTS = [(kx, b) for kx in (0, 2, 1) for b in (0, 1, 2)] + \
                [(0, 3), (2, 3), (1, 3)]

        def c2_rhs(b, r, kx, h):
            src = vs[b] if kx == 1 else vt[b]
            kxo = 0 if kx == 0 else 2
            return src[:, r, 4 * h:4 * h + 4, :, kxo:kxo + 14]

        for cob in range(4):
            ot = outp.tile([128, B_PER, 7, 2, 14], dt.int8, tag="ot", name="ot")
            for h in range(2):
                banks = [psum.tile([128, 512], dt.float32, tag="ps", name="ps")
                         for _ in range(4)]
                for r in range(4):
                    for si, (kx, b) in enumerate(SLOTS):
                        w_ap = u2_t[b][:, r * 3 + kx, cob, :]
                        mm(banks[r][:, 0:392], w_ap, c2_rhs(b, r, kx, h),
                           start=(si == 0), stop=(si == 11), reuse=False)
                for par, (ia, ib, ic, op1, op2) in enumerate(
                        ((0, 1, 2, Alu.add, Alu.add),
                         (1, 2, 3, Alu.subtract, Alu.subtract))):
                    # one PSUM operand per op: copy, then two accumulates
                    t0 = tmp.tile([128, 392], dt.float32, tag="wa", name="wa")
                    t1 = tmp.tile([128, 392], dt.float32, tag="wb", name="wb")
                    nc.vector.tensor_copy(t0[:], banks[ia][:, 0:392])
                    nc.vector.tensor_tensor(t1[:], t0[:], banks[ib][:, 0:392], op=op1)
                    nc.vector.tensor_tensor(t0[:], t1[:], banks[ic][:, 0:392], op=op2)
                    quant_chain(ot[:, 4 * h:4 * h + 4, :, par, :], t0[:],
                                scale2, b2_t[:, cob:cob + 1])
                nc.scalar.dma_start(
                    out=out_d[cob * 128:(cob + 1) * 128, 4 * h:4 * h + 4],
                    in_=ot[:, 4 * h:4 * h + 4].rearrange("p n i t w -> p n (i t w)"))

    _dedupe_ldweights(nc)
    nc.compile()
    return nc


def _dedupe_ldweights(nc):
    """Drop LDWEIGHTS whose stationary operand is identical to the previous
    one on the PE stream (only MATMULs in between): the PE array keeps its
    loaded weights, so consecutive same-weight matmuls need a single load."""
    def sig_of(inst):
        a0 = inst.ins[0]
        try:
            return (a0.memref, a0.offset, str(a0.ap), str(a0.dtype))
        except Exception:
            return None

    removed = 0
    for blk in nc.main_func.blocks:
        last = None
        keep = []
        for inst in blk.instructions:
            tn = type(inst).__name__
            if inst.engine == mybir.EngineType.PE:
                if tn == "InstLdweights":
                    sig = sig_of(inst)
                    si = inst.sync_info
                    clean = si is None or (not si.on_wait and not si.on_update)
                    if sig is not None and sig == last and clean:
                        removed += 1
                        continue
                    last = sig
                elif tn != "InstMatmult":
                    last = None
            keep.append(inst)
        blk.instructions[:] = keep
    return removed


def prepare(x, w1, b1, w2, b2, in_scale, act1_scale, act2_scale):
    """Host-side prep: quantize weights, build per-core input maps + immediates."""
    x = np.asarray(x, np.float32)
    w1 = np.asarray(w1, np.float32)
    b1 = np.asarray(b1, np.float32)
    w2 = np.asarray(w2, np.float32)
    b2 = np.asarray(b2, np.float32)
    s_in = np.float32(np.asarray(in_scale).reshape(-1)[0])
    s_a1 = np.float32(np.asarray(act1_scale).reshape(-1)[0])
    s_a2 = np.float32(np.asarray(act2_scale).reshape(-1)[0])

    w1_int, s_w1 = _quant_weights(w1)
    w2_int, s_w2 = _quant_weights(w2)
    bq1 = np.clip(np.round(b1 / (s_in * s_w1)), -2.0 ** 31, 2.0 ** 31 - 1).astype(np.float32) * (s_in * s_w1)
    bq2 = np.clip(np.round(b2 / (s_a1 * s_w2)), -2.0 ** 31, 2.0 ** 31 - 1).astype(np.float32) * (s_a1 * s_w2)

    scale1 = float(np.float32(s_w1 / s_a1))
    scale2 = float(np.float32(s_a1 * s_w2 / s_a2))
    out_scale = float(s_a2)
    bias1 = np.ascontiguousarray((bq1 / s_a1).astype(np.float32).reshape(4, 128).T)  # (128, 4)
    bias2 = np.ascontiguousarray((bq2 / s_a2).astype(np.float32).reshape(4, 128).T)

    xp_hi = _phase_planes(x).astype(np.float16)            # (64, 256, 4, 15, 16)

    # (9, 2, 128, 4, 128) -> taps reordered by TAP_ORDER, partition-major
    w1_l = np.ascontiguousarray(
        _w_lhsT(w1_int, 2)[TAP_ORDER].transpose(2, 0, 1, 3, 4)).astype(np.float16)
    # conv2 1D row-Winograd weights U[r, kx] = G-combos over ky (values k/2,
    # |k| <= 381: exact in fp16)
    g = w2_int.transpose(2, 3, 1, 0)                       # (ky, kx, ci, co)
    U = np.stack([g[0], (g[0] + g[1] + g[2]) * 0.5,
                  (g[0] - g[1] + g[2]) * 0.5, g[2]])       # (r4, kx3, ci, co)
    u = U.reshape(4, 3, 4, 128, 4, 128)                    # (r, kx, cib, ci, cob, co)
    u2_l = np.ascontiguousarray(
        u.transpose(2, 3, 0, 1, 4, 5)).astype(np.float16).reshape(4, 128, 12, 4, 128)

    in_maps = []
    for c in range(N_CORES):
        sl = slice(c * B_PER, (c + 1) * B_PER)
        # (8, 256, 4, 15, 16) -> (buf 4, ci_blk 2, ci_p 128, n 8, 15, 16)
        a = xp_hi[sl].transpose(2, 1, 0, 3, 4).reshape(4, 2, 128, B_PER, 15, 16)
        m = {"xhi": np.ascontiguousarray(a),
             "w1": w1_l, "u2": u2_l, "b1": bias1, "b2": bias2}
        in_maps.append(m)
    return (scale1, scale2, out_scale), in_maps


_OUT_SCALE = [np.float32(1.0)]


def gather_out(results):
    """Per-core (512, 8, 7, 2, 14) int8 outputs -> full (64, 512, 14, 14) fp32."""
    out = np.empty((N_CORES * B_PER, 512, 14, 14), np.float32)
    for c, r in enumerate(results):
        o = np.asarray(r["out"]).astype(np.float32).reshape(512, B_PER, 196)
        o *= _OUT_SCALE[0]
        out[c * B_PER:(c + 1) * B_PER] = o.transpose(1, 0, 2).reshape(B_PER, 512, 14, 14)
    return out


_cache = {}


def kernel(x, w1, b1, w2, b2, in_scale, act1_scale, act2_scale):
    imms, in_maps = prepare(x, w1, b1, w2, b2, in_scale, act1_scale, act2_scale)
    _OUT_SCALE[0] = np.float32(imms[2])
    if imms not in _cache:
        _cache[imms] = build_program(*imms)
    nc = _cache[imms]
    res = run_bass_kernel_spmd(nc, in_maps, list(range(N_CORES)))
    return gather_out(res.results)


# revision 76
# speedup vs baseline: 1.0073x; 1.0073x over previous
"""Trainium2 Bass kernel for the quantized BasicBlock (conv3x3/s2 + fakequant + conv3x3/s1 + fakequant).

Sharding: data-parallel over batch across 8 cores (8 images each), weights replicated.

Device math (per core, B=8):
  conv1: implicit GEMM, 9 taps x 2 ci-blocks, input as fp16 (11-bit significand,
         rel err ~4e-3 on final output vs 2e-2 budget), integer-valued fp16 weights
         (exact), fp32 PSUM accum.
  act1:  v = P1*(s_w1/s_a1) + bq1/s_a1; y = clip(rne(v), -128, 127) via the fp32
         magic-number trick on the DVE; y stored as integer-valued bf16 into a
         zero-padded [16x16] layout for conv2 (plus a 1-col-shifted copy, see below).
  conv2: exact integer bf16 GEMM, 9 taps x 4 ci-blocks.
  act2:  v2 = P2*(s_a1*s_w2/s_a2) + bq2/s_a2; out = clip(rne(v2), -128, 127) * s_a2.

Stride-2 conv1 is handled by a host-side phase split into 2x2 parity planes so each
tap reads a stride-1 14x14 window of one plane.

Alignment: the PE streams the moving operand ~14% slower when its base address is
not 4B-aligned. With 2-byte elements, tap windows at odd column offsets are slow.
Fix: every buffer is stored so all tap windows start at EVEN element offsets --
x planes come in 6 variants (2 normal + 4 one-col-shifted) prepared on host, and
act1 is written twice (normal + 1-col-shifted copy done by the idle Scalar engine).

DMAs are split into small per-(tap/plane, ci-block) chunks with their own tiles and
spread across the two HWDGE queues (Sync: x planes, Scalar: weights) in first-use
order, so the matmul stream starts as soon as tap 0's weights + plane arrive.
"""
import os
import sys
from contextlib import ExitStack

import numpy as np
import ml_dtypes

for _p in ("/opt/trn_rl_repo",):
    if _p not in sys.path and os.path.isdir(_p):
        sys.path.insert(0, _p)

import concourse.bacc as bacc
import concourse.tile as tile
import concourse.mybir as mybir
from concourse.bass_utils import run_bass_kernel_spmd

BF16 = ml_dtypes.bfloat16
N_CORES = 8
B_PER = 8           # images per core
MAGIC = float(np.float32(1.5 * 2 ** 23))   # fp32 RNE rounding magic
Alu = mybir.AluOpType
Act = mybir.ActivationFunctionType
dt = mybir.dt

# tap index k in {0,1,2} -> (parity s, window row offset) for the phase planes
_TAP = {0: (1, 0), 1: (0, 1), 2: (1, 1)}

# conv1 taps grouped by x-plane buffer (first-use order for the DMA ramp)
TAP_ORDER = [0, 6, 2, 8, 1, 7, 3, 5, 4]

# x plane buffers: (sr, sc, col offset of data); 0/1 normal sc=1 planes for kx=0
# (window c0=0), 2..5 one-col-shifted planes pl0..pl3 for kx=1/2 (window c0=2)
_XBUFS = [(0, 1, 1), (1, 1, 1), (0, 0, 2), (0, 1, 2), (1, 0, 2), (1, 1, 2)]


def _c1_src(t9):
    """tap -> (x buffer index, row offset r0, col offset c0); c0 always even."""
    ky, kx = divmod(t9, 3)
    sr, r0 = _TAP[ky]
    sc, _ = _TAP[kx]
    if kx == 0:
        return sr, r0, 0
    return 2 + sr * 2 + sc, r0, 2


# buffers 3 and 5 (shifted copies of 0 and 1) are built on-device by GpSimd
# shift-copies; only these 4 go over DMA (dram slot order):
_XDMA = [0, 1, 2, 4]
_XSLOT = {u: s for s, u in enumerate(_XDMA)}


def _phase_planes(x):
    """(B, C, 28, 28) f32 -> (B, C, 4, 15, 16) parity-plane buffers per _XDMA."""
    B, C = x.shape[:2]
    out = np.zeros((B, C, 4, 15, 16), np.float32)
    for s, u in enumerate(_XDMA):
        sr, sc, off = _XBUFS[u]
        out[:, :, s, 1:15, off:off + 14] = x[:, :, sr::2, sc::2]
    return out


def _quant_weights(w):
    """Per-tensor int8 narrow-range fake quant; returns (int-valued f32 weights, scale)."""
    s = np.float32(np.max(np.abs(w))) / np.float32(127.0)
    wq = np.clip(np.round(w / s), -127, 127).astype(np.float32)
    return wq, s


def _w_lhsT(w_int, n_ci_blk):
    """(Cout=512, Cin, 3, 3) int-valued -> (tap 9, ci_blk, 128, 4, 128) bf16 layout."""
    t = w_int.transpose(2, 3, 1, 0)                      # (3, 3, Cin, 512)
    t = t.reshape(9, n_ci_blk, 128, 4, 128)              # (tap, ci_blk, ci_p, co_blk, co)
    return np.ascontiguousarray(t).astype(BF16)


_skip_ldw = [False]
_orig_InstMatmult = mybir.InstMatmult


def _patched_InstMatmult(*a, **kw):
    if _skip_ldw[0]:
        kw.setdefault("ldweights", False)
    return _orig_InstMatmult(*a, **kw)


def build_program(scale1, scale2, out_scale):
    """Build the (per-core SPMD) Bass program with the given fp32 immediates."""
    nc = bacc.Bacc("TRN2", target_bir_lowering=False, debug=False,
                   num_devices=N_CORES)

    mybir.InstMatmult = _patched_InstMatmult
    try:
        return _build_body(nc, scale1, scale2, out_scale)
    finally:
        mybir.InstMatmult = _orig_InstMatmult


def _build_body(nc, scale1, scale2, out_scale):
    NT = 4

    xhi_d = nc.dram_tensor("xhi", (4, 2, 128, B_PER, 15, 16), dt.float16, kind="ExternalInput")
    # w1 packed partition-major with taps pre-ordered by TAP_ORDER
    w1_d = nc.dram_tensor("w1", (128, 9, 2, 4, 128), dt.float16, kind="ExternalInput")
    # conv2 weights as 1D row-Winograd U (4 r-positions x 3 kx taps), fp16 exact
    u2_d = nc.dram_tensor("u2", (4, 128, 12, 4, 128), dt.float16, kind="ExternalInput")
    b1_d = nc.dram_tensor("b1", (128, 4), dt.float32, kind="ExternalInput")
    b2_d = nc.dram_tensor("b2", (128, 4), dt.float32, kind="ExternalInput")
    out_d = nc.dram_tensor("out", (512, B_PER, 196), dt.int8, kind="ExternalOutput")

    def mm(out_ap, w_ap, rhs, start, stop, reuse):
        # reuse=True -> PE keeps the already-loaded stationary weights
        _skip_ldw[0] = reuse
        try:
            nc.tensor.matmul(out_ap, w_ap, rhs, start=start, stop=stop)
        finally:
            _skip_ldw[0] = False

    with tile.TileContext(nc) as tc, ExitStack() as ctx:
        const = ctx.enter_context(tc.tile_pool(name="const", bufs=1))
        psum = ctx.enter_context(tc.tile_pool(name="psum", bufs=8, space="PSUM"))
        tmp = ctx.enter_context(tc.tile_pool(name="tmp", bufs=3))
        outp = ctx.enter_context(tc.tile_pool(name="outp", bufs=3))

        # --- SBUF allocations: one tile per DMA chunk for fine-grained deps ---
        xb = {(u, b): const.tile([128, B_PER, 15, 16], dt.float16,
                                 tag=f"x{u}_{b}", name=f"x{u}_{b}")
              for u in range(6) for b in range(2)}
        # w1 grouped along TAP_ORDER; first two taps get their own chunk so the
        # very first matmul's weights arrive (and are usable, fp16, no cast) ASAP
        W1G = [(0, 1), (1, 2), (2, 4), (4, 6), (6, 9)]
        w1f_g = [const.tile([128, g1 - g0, 2, 4, 128], dt.float16,
                            tag=f"w1f{g0}", name=f"w1f{g0}") for g0, g1 in W1G]

        def w1_ap(t, b, cb):
            p = TAP_ORDER.index(t)
            for gi, (g0, g1) in enumerate(W1G):
                if g0 <= p < g1:
                    return w1f_g[gi][:, p - g0, b, cb, :]

        u2_t = [const.tile([128, 12, 4, 128], dt.float16, tag=f"u2{b}", name=f"u2t{b}")
                for b in range(4)]
        b1_t = const.tile([128, 4], dt.float32, tag="b1")
        b2_t = const.tile([128, 4], dt.float32, tag="b2")
        act_t = const.tile([128, 4, B_PER, 16, 16], dt.bfloat16, tag="act")   # data cols 1..14
        # 1D row-transformed act (V): [cib] -> [128, r4, img, i7, 16cols], fp16
        # exact (|V| <= 255); vs = 1-col-shifted copy for the kx=1 taps
        vt = [const.tile([128, 4, B_PER, 7, 16], dt.float16, tag=f"vt{b}", name=f"vt{b}")
              for b in range(4)]
        vs = [const.tile([128, 4, B_PER, 7, 16], dt.float16, tag=f"vs{b}", name=f"vs{b}")
              for b in range(4)]
        gate2 = const.tile([128, 1], dt.bfloat16, tag="g2")
        wz = const.tile([128, 256], dt.bfloat16, tag="wz")

        # PE warm-up source + conv2 padding zeros on the otherwise idle GpSimd
        nc.vector.memset(wz[:], 0.0)
        nc.gpsimd.memset(act_t[:], 0.0)

        # --- loads interleaved across both HWDGE queues in demand order so the
        # conv1 tap stream (one 8-MM group per ~0.67us) never outruns delivery.
        # Engine order IS execution order (FIFO queues), so everything below is
        # sequenced explicitly by first use. ---
        def lx(q, u, b):
            q(out=xb[(u, b)][:], in_=xhi_d[_XSLOT[u], b])

        sy, sc_q = nc.sync.dma_start, nc.scalar.dma_start
        lx(sy, 1, 0); lx(sy, 1, 1); lx(sy, 4, 0); lx(sy, 0, 0); lx(sy, 2, 0)
        for gi, (g0, g1) in enumerate(W1G):
            sc_q(out=w1f_g[gi][:], in_=w1_d[:, g0:g1])
            if gi == 3:
                lx(sc_q, 4, 1)
        lx(sc_q, 0, 1)
        sc_q(out=b1_t[:], in_=b1_d[:])
        lx(sc_q, 2, 1)

        # DVE: shifted x duplicates (buf5 <- buf1, buf3 <- buf0). The zero pad
        # ring makes a FLAT +1-element shift exactly equal to the per-row
        # column shift, so this is a fast contiguous 2-byte copy.
        def xshift(u_dst, u_src, b, eng):
            df = xb[(u_dst, b)][:].rearrange("p a b c -> p (a b c)")
            sf = xb[(u_src, b)][:].rearrange("p a b c -> p (a b c)")
            if eng == "v":
                nc.vector.tensor_copy(df[:, 1:1920], sf[:, 0:1919])
            else:
                nc.scalar.activation(df[:, 1:1920], sf[:, 0:1919], Act.Copy)

        # first copy on the DVE (free immediately); the rest on the Scalar
        # engine, which finishes its DMA triggers around the time they're due
        xshift(5, 1, 0, "v")
        xshift(5, 1, 1, "s")
        xshift(3, 0, 0, "s")
        xshift(3, 0, 1, "s")

        def quant_chain(dst, src, sc, bias_ap, width=392):
            """dst = clip(rne(src*sc + bias), -128, 127) on the DVE (3 fused ops)."""
            tt = tmp.tile([128, width], dt.float32, tag=f"tt{min(width, 392)}", name="tt")
            nc.vector.tensor_scalar(tt[:], src, sc, bias_ap, op0=Alu.mult, op1=Alu.add)
            nc.vector.tensor_scalar(tt[:], tt[:], MAGIC, MAGIC + 127.0, op0=Alu.add, op1=Alu.min)
            nc.vector.tensor_scalar(dst, tt[:], MAGIC - 128.0, -MAGIC, op0=Alu.max, op1=Alu.add)
            return tt

        # PE warm-up: junk matmuls on the zeroed tile during the input-DMA wait
        # so the HAM clock gate is at full rate when the real stream starts.
        wps = psum.tile([128, 512], dt.float32, tag="ps", name="warmps")
        for i in range(16):
            nc.tensor.matmul(wps[:, 0:256], wz[:, 0:128], wz[:, 0:256],
                             start=True, stop=True)

        # --- conv1 + act1 ---
        def conv1_group(cb, t9, b, ps_list, nts):
            # one stationary weight (t9, b, cb) serving len(nts) matmuls;
            # only the first self-loads the PE array
            u, r0, c0 = _c1_src(t9)
            w_ap = w1_ap(t9, b, cb)
            for i, nt in enumerate(nts):
                rhs = xb[(u, b)][:, 2 * nt:2 * nt + 2, r0:r0 + 14, c0:c0 + 14]
                mm(ps_list[i][:, 0:392], w_ap, rhs,
                   start=(t9 == TAP_ORDER[0] and b == 0),
                   stop=(t9 == TAP_ORDER[-1] and b == 1),
                   reuse=i > 0)

        def act1_store(cb, nt, ps):
            quant_chain(act_t[:, cb, 2 * nt:2 * nt + 2, 1:15, 1:15],
                        ps[:, 0:392], scale1, b1_t[:, cb:cb + 1])

        def v_transform(b, h, eng):
            """Row-stage Winograd transform of act cib b, image half h (DVE),
            then a flat +1-shift copy into vs (pad ring makes it exact)."""
            im = slice(4 * h, 4 * h + 4)
            d = [act_t[:, b, im, k:k + 13:2, :] for k in range(4)]  # rows 2i+k, [4,7,16]
            v = nc.vector
            v.tensor_tensor(vt[b][:, 0, im], d[0], d[2], op=Alu.subtract)
            v.tensor_tensor(vt[b][:, 1, im], d[1], d[2], op=Alu.add)
            v.tensor_tensor(vt[b][:, 2, im], d[2], d[1], op=Alu.subtract)
            v.tensor_tensor(vt[b][:, 3, im], d[1], d[3], op=Alu.subtract)
            for r in range(4):
                df = vs[b][:, r, im].rearrange("p a b c -> p (a b c)")
                sf = vt[b][:, r, im].rearrange("p a b c -> p (a b c)")
                if eng == "v":
                    nc.vector.tensor_copy(df[:, 1:448], sf[:, 0:447])
                else:
                    nc.scalar.activation(df[:, 1:448], sf[:, 0:447], Act.Copy)

        for cb in range(4):
            if cb == 0:
                # tap-major: plane demand spread over the whole group to match
                # the DMA delivery ramp; 8 matmuls per weight load
                ps_n = [psum.tile([128, 512], dt.float32, tag="ps", name="ps")
                        for _ in range(NT)]
                for t9 in TAP_ORDER:
                    for b in range(2):
                        conv1_group(cb, t9, b, ps_n, range(NT))
                for nt in range(NT):
                    act1_store(cb, nt, ps_n[nt])
                # V transform for cb0; its Scalar-engine shift copies double as
                # the gate that defers the u2 load past the x/w1 ramp
                v_transform(0, 0, "s")
                v_transform(0, 1, "s")
                for b in range(4):
                    sc_q(out=u2_t[b][:], in_=u2_d[b])
                sc_q(out=b2_t[:], in_=b2_d[:])
            else:
                # nt-pair-major: each bank pair finishes at half-time so its
                # epilogue overlaps the rest; 4 matmuls per weight load
                for half in range(2):
                    nts = [2 * half, 2 * half + 1]
                    ps_p = [psum.tile([128, 512], dt.float32, tag="ps", name="ps")
                            for _ in nts]
                    for t9 in TAP_ORDER:
                        for b in range(2):
                            conv1_group(cb, t9, b, ps_p, nts)
                    for i, nt in enumerate(nts):
                        act1_store(cb, nt, ps_p[i])
                    # cb3 (needed first ~1.5us into conv2) runs on the DVE right
                    # behind its own epilogue; earlier cbs go to Scalar
                    v_transform(cb, half, "v" if cb == 3 else "s")

        # --- conv2 via 1D row-Winograd + act2 ---
        # out rows pairs: even = M0+M1+M2, odd = M1-M2-M3 over the 4 r-banks.
        # Per (cob, half): 4 banks x 12 accumulating MMs over (kx, cib);
        # cib 3 (conv1's last output block) is ordered last so conv2 can start
        # before conv1's tail epilogue + V transform fully drain.
        SLOTS = [(kx, b) for kx in (0, 2, 1) for b in (0, 1, 2)] + \
                [(0, 3), (2, 3), (1, 3)]

        def c2_rhs(b, r, kx, h):
            src = vs[b] if kx == 1 else vt[b]
            kxo = 0 if kx == 0 else 2
            return src[:, r, 4 * h:4 * h + 4, :, kxo:kxo + 14]

        for cob in range(4):
            ot = outp.tile([128, B_PER, 7, 2, 14], dt.int8, tag="ot", name="ot")
            for h in range(2):
                banks = [psum.tile([128, 512], dt.float32, tag="ps", name="ps")
                         for _ in range(4)]
                for r in range(4):
                    for si, (kx, b) in enumerate(SLOTS):
                        w_ap = u2_t[b][:, r * 3 + kx, cob, :]
                        mm(banks[r][:, 0:392], w_ap, c2_rhs(b, r, kx, h),
                           start=(si == 0), stop=(si == 11), reuse=False)
                for par, (ia, ib, ic, op1, op2) in enumerate(
                        ((0, 1, 2, Alu.add, Alu.add),
                         (1, 2, 3, Alu.subtract, Alu.subtract))):
                    # one PSUM operand per op: copy, then two accumulates
                    t0 = tmp.tile([128, 392], dt.float32, tag="wa", name="wa")
                    t1 = tmp.tile([128, 392], dt.float32, tag="wb", name="wb")
                    nc.vector.tensor_copy(t0[:], banks[ia][:, 0:392])
                    nc.vector.tensor_tensor(t1[:], t0[:], banks[ib][:, 0:392], op=op1)
                    nc.vector.tensor_tensor(t0[:], t1[:], banks[ic][:, 0:392], op=op2)
                    quant_chain(ot[:, 4 * h:4 * h + 4, :, par, :], t0[:],
                                scale2, b2_t[:, cob:cob + 1])
                nc.scalar.dma_start(
                    out=out_d[cob * 128:(cob + 1) * 128, 4 * h:4 * h + 4],
                    in_=ot[:, 4 * h:4 * h + 4].rearrange("p n i t w -> p n (i t w)"))

    _dedupe_ldweights(nc)
    nc.compile()
    return nc


def _dedupe_ldweights(nc):
    """Drop LDWEIGHTS whose stationary operand is identical to the previous
    one on the PE stream (only MATMULs in between): the PE array keeps its
    loaded weights, so consecutive same-weight matmuls need a single load."""
    def sig_of(inst):
        a0 = inst.ins[0]
        try:
            return (a0.memref, a0.offset, str(a0.ap), str(a0.dtype))
        except Exception:
            return None

    removed = 0
    for blk in nc.main_func.blocks:
        last = None
        keep = []
        for inst in blk.instructions:
            tn = type(inst).__name__
            if inst.engine == mybir.EngineType.PE:
                if tn == "InstLdweights":
                    sig = sig_of(inst)
                    si = inst.sync_info
                    clean = si is None or (not si.on_wait and not si.on_update)
                    if sig is not None and sig == last and clean:
                        removed += 1
                        continue
                    last = sig
                elif tn != "InstMatmult":
                    last = None
            keep.append(inst)
        blk.instructions[:] = keep
    return removed


def prepare(x, w1, b1, w2, b2, in_scale, act1_scale, act2_scale):
    """Host-side prep: quantize weights, build per-core input maps + immediates."""
    x = np.asarray(x, np.float32)
    w1 = np.asarray(w1, np.float32)
    b1 = np.asarray(b1, np.float32)
    w2 = np.asarray(w2, np.float32)
    b2 = np.asarray(b2, np.float32)
    s_in = np.float32(np.asarray(in_scale).reshape(-1)[0])
    s_a1 = np.float32(np.asarray(act1_scale).reshape(-1)[0])
    s_a2 = np.float32(np.asarray(act2_scale).reshape(-1)[0])

    w1_int, s_w1 = _quant_weights(w1)
    w2_int, s_w2 = _quant_weights(w2)
    bq1 = np.clip(np.round(b1 / (s_in * s_w1)), -2.0 ** 31, 2.0 ** 31 - 1).astype(np.float32) * (s_in * s_w1)
    bq2 = np.clip(np.round(b2 / (s_a1 * s_w2)), -2.0 ** 31, 2.0 ** 31 - 1).astype(np.float32) * (s_a1 * s_w2)

    scale1 = float(np.float32(s_w1 / s_a1))
    scale2 = float(np.float32(s_a1 * s_w2 / s_a2))
    out_scale = float(s_a2)
    bias1 = np.ascontiguousarray((bq1 / s_a1).astype(np.float32).reshape(4, 128).T)  # (128, 4)
    bias2 = np.ascontiguousarray((bq2 / s_a2).astype(np.float32).reshape(4, 128).T)

    xp_hi = _phase_planes(x).astype(np.float16)            # (64, 256, 4, 15, 16)

    # (9, 2, 128, 4, 128) -> taps reordered by TAP_ORDER, partition-major
    w1_l = np.ascontiguousarray(
        _w_lhsT(w1_int, 2)[TAP_ORDER].transpose(2, 0, 1, 3, 4)).astype(np.float16)
    # conv2 1D row-Winograd weights U[r, kx] = G-combos over ky (values k/2,
    # |k| <= 381: exact in fp16)
    g = w2_int.transpose(2, 3, 1, 0)                       # (ky, kx, ci, co)
    U = np.stack([g[0], (g[0] + g[1] + g[2]) * 0.5,
                  (g[0] - g[1] + g[2]) * 0.5, g[2]])       # (r4, kx3, ci, co)
    u = U.reshape(4, 3, 4, 128, 4, 128)                    # (r, kx, cib, ci, cob, co)
    u2_l = np.ascontiguousarray(
        u.transpose(2, 3, 0, 1, 4, 5)).astype(np.float16).reshape(4, 128, 12, 4, 128)

    in_maps = []
    for c in range(N_CORES):
        sl = slice(c * B_PER, (c + 1) * B_PER)
        # (8, 256, 4, 15, 16) -> (buf 4, ci_blk 2, ci_p 128, n 8, 15, 16)
        a = xp_hi[sl].transpose(2, 1, 0, 3, 4).reshape(4, 2, 128, B_PER, 15, 16)
        m = {"xhi": np.ascontiguousarray(a),
             "w1": w1_l, "u2": u2_l, "b1": bias1, "b2": bias2}
        in_maps.append(m)
    return (scale1, scale2, out_scale), in_maps


_OUT_SCALE = [np.float32(1.0)]


def gather_out(results):
    """Per-core (512, 8, 7, 2, 14) int8 outputs -> full (64, 512, 14, 14) fp32."""
    out = np.empty((N_CORES * B_PER, 512, 14, 14), np.float32)
    for c, r in enumerate(results):
        o = np.asarray(r["out"]).astype(np.float32).reshape(512, B_PER, 196)
        o *= _OUT_SCALE[0]
        out[c * B_PER:(c + 1) * B_PER] = o.transpose(1, 0, 2).reshape(B_PER, 512, 14, 14)
    return out


_cache = {}


def kernel(x, w1, b1, w2, b2, in_scale, act1_scale, act2_scale):
    imms, in_maps = prepare(x, w1, b1, w2, b2, in_scale, act1_scale, act2_scale)
    _OUT_SCALE[0] = np.float32(imms[2])
    if imms not in _cache:
        _cache[imms] = build_program(*imms)
    nc = _cache[imms]
    res = run_bass_kernel_spmd(nc, in_maps, list(range(N_CORES)))
    return gather_out(res.results)
